# revision 17
# baseline (speedup 1.0000x reference)
"""Trainium2 Bass kernel for nn_PlannerIQGP (vq_codebook arch).

Sharding: data-parallel over batch — core b computes batch element b
end-to-end (no collectives). All matmuls run in fp16 (PE at full rate);
elementwise/softmax/LN/argmax run in fp32. Transposed operands (weights,
activations entering projections) are produced by an on-chip fp32->fp16
cast pass to DRAM scratch followed by XBAR DMA-transpose loads. Biases are
folded into the matmul accumulation as K=1 rank-1 updates.

kernel(**inputs) takes the FULL unsharded inputs and returns the full
outputs as a tuple matching reference.py.
"""
import os
import sys
sys.path.insert(0, '/opt/trn_rl_repo')
import numpy as np

P = 128
FULL_DIMS = dict(B=8, S=1024, H=1024, NH=8, N=256, K=8192, E=2048, U=512)

_BUILD_CACHE = {}
LAST_RESULT = None


def build_program(B=8, S=1024, H=1024, NH=8, N=256, K=8192, E=2048, U=512):
    import concourse.bacc as bacc
    import concourse.bass as bass
    import concourse.tile as tile
    import concourse.mybir as mybir

    F32 = mybir.dt.float32
    F16 = mybir.dt.float16
    U32 = mybir.dt.uint32
    I32 = mybir.dt.int32
    U8 = mybir.dt.uint8
    AF = mybir.ActivationFunctionType
    OP = mybir.AluOpType

    HD = H // NH
    assert HD == 128
    HN, SN, NN, KN = H // P, S // P, N // P, K // P
    H3 = 3 * H
    INV_SQRT_HD = float(1.0 / np.sqrt(HD))

    nc = bacc.Bacc("TRN2", target_bir_lowering=False, debug=False)

    d = {}
    def din(name, shape, dt=F32):
        d[name] = nc.dram_tensor(name, shape, dt, kind="ExternalInput")
    def dout(name, shape, dt=F32):
        d[name] = nc.dram_tensor(name, shape, dt, kind="ExternalOutput")

    din("h_en", [S, H]); din("h_zh", [S, H])
    din("ent", [E, H]); din("unit", [U, H])
    for p_ in ("en", "zh", "nd"):
        din(p_ + "_in_w", [H3, H]); din(p_ + "_in_b", [H3])
        din(p_ + "_out_w", [H, H]); din(p_ + "_out_b", [H])
    din("fuse_w1", [H, 2 * H]); din("fuse_b1", [H])
    din("fuse_w2", [H, H]); din("fuse_b2", [H])
    din("node_queries", [N, H]); din("norm_g", [H]); din("norm_b", [H])
    din("codebook", [K, H])
    din("ent_ptr_w", [H, H]); din("unit_ptr_w", [H, H])
    din("edge_w", [H, H]); din("edge_b", [1]); din("edge_bias", [N, N])
    din("len_w1", [H, H]); din("len_b1", [H]); din("len_w2", [H]); din("len_b2", [1])

    dout("quantized", [N, H])
    dout("codes", [N], I32)
    dout("qid_logits", [N, E])
    dout("unit_logits", [N, U])
    dout("edge_logits", [N, N])
    dout("node_mask", [N], U8)
    dout("vq_partial", [1, 1])

    # fp16 scratch in DRAM (natural layouts; transposed reads via XBAR)
    sc = {}
    for nm, shp in [("h_en", [S, H]), ("h_zh", [S, H]),
                    ("in_w_en", [H3, H]), ("in_w_zh", [H3, H]), ("in_w_nd", [H3, H]),
                    ("out_w_en", [H, H]), ("out_w_zh", [H, H]), ("out_w_nd", [H, H]),
                    ("fw1", [H, 2 * H]), ("fw2", [H, H]), ("lw1", [H, H]),
                    ("pw_ent", [H, H]), ("pw_unit", [H, H]),
                    ("ent", [E, H]), ("unit", [U, H]), ("cb", [K, H]),
                    ("nq", [N, H]), ("nodes", [N, H]), ("qz", [N, H])]:
        sc[nm] = nc.dram_tensor("sc_" + nm, shp, F16)
    sc_vqsum = nc.dram_tensor("sc_vqsum", [P], F32)

    with tile.TileContext(nc) as tc:
      with tc.tile_pool(name="const", bufs=1) as const, \
           tc.tile_pool(name="wst", bufs=2) as wst, \
           tc.tile_pool(name="psA", bufs=3, space="PSUM") as psA, \
           tc.tile_pool(name="psS", bufs=2, space="PSUM") as psS:

        def open_pool(name, bufs=1, side="left"):
            cm = tc.tile_pool(name=name, bufs=bufs, side=side)
            return cm, cm.__enter__()

        def close_pool(cm):
            cm.__exit__(None, None, None)

        ones_row = const.tile([1, 512], F16)
        nc.vector.memset(ones_row[:], 1.0)
        ones_col = const.tile([P, 1], F16)
        nc.vector.memset(ones_col[:], 1.0)
        zero_row = const.tile([1, 512], F16)
        nc.vector.memset(zero_row[:], 0.0)
        eps_t = const.tile([P, 1], F32)
        nc.vector.memset(eps_t[:], 1e-5)

        # ---------------- small helpers ----------------
        def frow(vec_ap, D, dt=F16, name="r", pool=None):
            """1-D [D] fp32 input -> [1, D] tile in dtype dt."""
            pool = pool or const
            t32 = p_cast.tile([1, D], F32, tag="frow32", name="fr32")
            nc.sync.dma_start(t32[:], vec_ap.rearrange("(a b) -> a b", a=1))
            if dt == F32:
                t32k = pool.tile([1, D], F32, tag="frow32k_" + name, name="fr32k")
                nc.vector.tensor_copy(t32k[:], t32[:])
                return t32k
            t16 = pool.tile([1, D], dt, tag="frow16_" + name, name="fr16")
            nc.vector.tensor_copy(t16[:], t32[:])
            return t16

        _cast_rr = [0]
        def cast_pass(src_ap, R, C, dst):
            """fp32 [R, C] DRAM -> fp16 scratch, engines round-robined."""
            engs = [nc.vector, nc.vector, nc.gpsimd, nc.scalar]
            for rc in range(R // P):
                nat = p_cast.tile([P, C], F32, tag="cp_nat", name="cp_nat")
                nc.sync.dma_start(nat[:], src_ap[rc * P:(rc + 1) * P, :])
                c16 = p_cast.tile([P, C], F16, tag="cp_f16", name="cp_f16")
                eng = engs[_cast_rr[0] % 4]; _cast_rr[0] += 1
                if eng is nc.scalar:
                    nc.scalar.copy(c16[:], nat[:])
                else:
                    eng.tensor_copy(c16[:], nat[:])
                nc.sync.dma_start(dst.ap()[rc * P:(rc + 1) * P, :], c16[:])

        def ldT(scr, r0, nr, c0):
            """[128, nr] f16 tile = transpose of scratch[r0:r0+nr, c0:c0+128]."""
            t = wst.tile([P, nr], F16, tag="wst%d" % nr)
            nc.sync.dma_start_transpose(t[:], scr.ap()[r0:r0 + nr, c0:c0 + P])
            return t

        def fslices(D):
            if D <= 512:
                return [(0, D)]
            assert D % 512 == 0
            return [(i * 512, 512) for i in range(D // 512)]

        def proj(n_k, lhsT_fn, rhs_fn, fsl, bias_fn, epi_fn):
            """psum[i] = sum_k lhsT(k).T @ rhs(k,i)  (+ bias rank-1), then epi."""
            pss = []
            for (f0, fl) in fsl:
                pt = psA.tile([P, 512], F32, tag="a", name="pj")
                pss.append(pt[:, :fl])
            for k in range(n_k):
                lh = lhsT_fn(k)
                for i, (f0, fl) in enumerate(fsl):
                    nc.tensor.matmul(pss[i], lh, rhs_fn(k, f0, fl),
                                     start=(k == 0), stop=False)
            for i, (f0, fl) in enumerate(fsl):
                bl, br = bias_fn(f0, fl)
                nc.tensor.matmul(pss[i], bl, br, start=False, stop=True)
                epi_fn(i, f0, fl, pss[i])

        # ---------------- phase 0: rows + casts ----------------
        p_cast_cm, p_cast = open_pool("p_cast", bufs=3)
        b_in = {p_: frow(d[p_ + "_in_b"].ap(), H3, name=p_ + "ib") for p_ in ("en", "zh", "nd")}
        b_out = {p_: frow(d[p_ + "_out_b"].ap(), H, name=p_ + "ob") for p_ in ("en", "zh", "nd")}
        fb1_r = frow(d["fuse_b1"].ap(), H, name="fb1")
        fb2_r = frow(d["fuse_b2"].ap(), H, name="fb2")
        fb2x2 = const.tile([1, H], F16)
        nc.vector.tensor_scalar_mul(fb2x2[:], fb2_r[:], 2.0)
        lb1_r = frow(d["len_b1"].ap(), H, name="lb1")
        def bcast128(handle, D, name, pool):
            t = pool.tile([P, D], F32, tag="bc_" + name, name="bc")
            src_ap = bass.AP(tensor=handle, offset=0, ap=[[0, P], [1, D]])
            nc.gpsimd.dma_start(out=t[:], in_=src_ap)
            return t
        eb_128 = bcast128(d["edge_b"], 1, "eb", const)
        b2_128 = bcast128(d["len_b2"], 1, "b2", const)
        nb2neg = const.tile([P, 1], F32)
        nc.vector.tensor_scalar_mul(nb2neg[:], b2_128[:], -1.0)

        cast_pass(d["h_en"].ap(), S, H, sc["h_en"])
        cast_pass(d["h_zh"].ap(), S, H, sc["h_zh"])
        for p_ in ("en", "zh", "nd"):
            cast_pass(d[p_ + "_in_w"].ap(), H3, H, sc["in_w_" + p_])
            cast_pass(d[p_ + "_out_w"].ap(), H, H, sc["out_w_" + p_])
        cast_pass(d["fuse_w1"].ap(), H, 2 * H, sc["fw1"])
        cast_pass(d["fuse_w2"].ap(), H, H, sc["fw2"])
        cast_pass(d["len_w1"].ap(), H, H, sc["lw1"])
        cast_pass(d["ent_ptr_w"].ap(), H, H, sc["pw_ent"])
        cast_pass(d["unit_ptr_w"].ap(), H, H, sc["pw_unit"])
        cast_pass(d["ent"].ap(), E, H, sc["ent"])
        cast_pass(d["unit"].ap(), U, H, sc["unit"])
        cast_pass(d["codebook"].ap(), K, H, sc["cb"])
        cast_pass(d["node_queries"].ap(), N, H, sc["nq"])

        def loadT_full(scr, D2, SQ, tag, pool):
            """[128, D2//128, SQ] f16: full transposed activation."""
            t = pool.tile([P, D2 // P, SQ], F16, tag=tag, name=tag)
            for k in range(D2 // P):
                nc.sync.dma_start_transpose(t[:, k, :], scr.ap()[:, k * P:(k + 1) * P])
            return t

        close_pool(p_cast_cm)
        p_x_cm, p_x = open_pool("p_x")
        p_gate_cm, p_gate = open_pool("p_gate")
        p_attn_cm, p_attn = open_pool("p_attn")
        xT = {"en": loadT_full(sc["h_en"], H, S, "xT_en", p_x),
              "zh": loadT_full(sc["h_zh"], H, S, "xT_zh", p_x)}
        nqT = loadT_full(sc["nq"], H, N, "nqT", p_x)

        # ---------------- generic MHA ----------------
        def mha(xqT, SQ, xkvT, w_sc, ow_sc, b_row, ob_row, out_mode, out_tag, tpool, opool):
            fsl_q = fslices(SQ)
            fsl_h = fslices(H)
            # v natural [s, dout]
            v = tpool.tile([P, SN, H], F16, tag="m_v", name="m_v")
            for si in range(SN):
                proj(HN,
                     lambda k, si=si: xkvT[:, k, si * P:(si + 1) * P],
                     lambda k, f0, fl: ldT(w_sc, 2 * H + f0, fl, k * P),
                     fsl_h,
                     lambda f0, fl: (ones_row[0:1, :P], b_row[0:1, 2 * H + f0:2 * H + f0 + fl]),
                     lambda i, f0, fl, ps, si=si: nc.scalar.copy(v[:, si, f0:f0 + fl], ps))

            oT = tpool.tile([P, HN, SQ], F16, tag="m_oT", name="m_oT")
            qTh = tpool.tile([P, SQ], F16, tag="m_qTh", name="m_qTh")
            kTh = tpool.tile([P, S], F16, tag="m_kTh", name="m_kTh")
            for h in range(NH):
                proj(HN,
                     lambda k, h=h: ldT(w_sc, h * P, P, k * P),
                     lambda k, f0, fl: xqT[:, k, f0:f0 + fl],
                     fsl_q,
                     lambda f0, fl, h=h: (b_row[0:1, h * P:(h + 1) * P], ones_row[0:1, :fl]),
                     lambda i, f0, fl, ps: nc.scalar.copy(qTh[:, f0:f0 + fl], ps))
                proj(HN,
                     lambda k, h=h: ldT(w_sc, H + h * P, P, k * P),
                     lambda k, f0, fl: xkvT[:, k, f0:f0 + fl],
                     fslices(S),
                     lambda f0, fl, h=h: (b_row[0:1, H + h * P:H + (h + 1) * P], ones_row[0:1, :fl]),
                     lambda i, f0, fl, ps: nc.scalar.copy(kTh[:, f0:f0 + fl], ps))
                # per sq-half: scoresT+exp, denom, PV, normalize
                for (f0, fl) in fsl_q:
                    attnT = tpool.tile([P, SN, 512], F16, tag="m_attnT", name="m_attnT")
                    rden = tpool.tile([1, 512], F32, tag="m_rden", name="m_rden")
                    rden16 = tpool.tile([1, 512], F16, tag="m_rden16", name="m_rd16")
                    for c in range(SN):
                        pfull = psA.tile([P, 512], F32, tag="a", name="pf")
                        ps = pfull[:, :fl]
                        nc.tensor.matmul(ps, kTh[:, c * P:(c + 1) * P], qTh[:, f0:f0 + fl])
                        nc.scalar.activation(attnT[:, c, :fl], ps, AF.Exp,
                                             scale=INV_SQRT_HD)
                    psdf = psS.tile([1, 512], F32, tag="s", name="psd")
                    psd = psdf[:, :fl]
                    for c in range(SN):
                        nc.tensor.matmul(psd, ones_col[:, 0:1], attnT[:, c, :fl],
                                         start=(c == 0), stop=(c == SN - 1))
                    nc.vector.reciprocal(rden[0:1, :fl], psd)
                    nc.vector.tensor_copy(rden16[0:1, :fl], rden[0:1, :fl])
                    pfull = psA.tile([P, 512], F32, tag="a", name="po")
                    ps = pfull[:, :fl]
                    for c in range(SN):
                        nc.tensor.matmul(ps, v[:, c, h * P:(h + 1) * P],
                                         attnT[:, c, :fl],
                                         start=(c == 0), stop=(c == SN - 1))
                    pbf = psA.tile([P, 512], F32, tag="a", name="pb")
                    pb = pbf[:, :fl]
                    nc.tensor.matmul(pb, ones_row[0:1, :P], rden16[0:1, :fl])
                    rbc = tpool.tile([P, 512], F32, tag="m_rbc", name="m_rbc")
                    nc.scalar.copy(rbc[:, :fl], pb)
                    nc.vector.tensor_tensor(oT[:, h, f0:f0 + fl], ps, rbc[:, :fl], OP.mult)
            # out-projection
            if out_mode == "T":
                outT = opool.tile([P, HN, SQ], F16, tag=out_tag + "_outT", name="m_outT")
                for m in range(HN):
                    proj(HN,
                         lambda k, m=m: ldT(ow_sc, m * P, P, k * P),
                         lambda k, f0, fl: oT[:, k, f0:f0 + fl],
                         fsl_q,
                         lambda f0, fl, m=m: (ob_row[0:1, m * P:(m + 1) * P], ones_row[0:1, :fl]),
                         lambda i, f0, fl, ps, m=m: nc.scalar.copy(outT[:, m, f0:f0 + fl], ps))
                return outT
            else:
                out_nat = opool.tile([P, SQ // P, H], F32, tag=out_tag + "_nat", name="m_out_nat")
                for nn2 in range(SQ // P):
                    proj(HN,
                         lambda k, nn2=nn2: oT[:, k, nn2 * P:(nn2 + 1) * P],
                         lambda k, f0, fl: ldT(ow_sc, f0, fl, k * P),
                         fsl_h,
                         lambda f0, fl: (ones_row[0:1, :P], ob_row[0:1, f0:f0 + fl]),
                         lambda i, f0, fl, ps, nn2=nn2: nc.vector.tensor_copy(out_nat[:, nn2, f0:f0 + fl], ps))
                return out_nat

        # ---------------- cross attentions ----------------
        p_mc_cm, p_mc = open_pool("p_mc", side="right")
        attn_enT = mha(xT["en"], S, xT["zh"], sc["in_w_en"], sc["out_w_en"],
                       b_in["en"], b_out["en"], "T", "aen", p_mc, p_attn)
        attn_zhT = mha(xT["zh"], S, xT["en"], sc["in_w_zh"], sc["out_w_zh"],
                       b_in["zh"], b_out["zh"], "T", "azh", p_mc, p_attn)
        close_pool(p_mc_cm)

        # ---------------- gates + fuse ----------------
        fsl_s = fslices(S)
        fstage = p_gate.tile([P, HN, S], F16, tag="fstage", name="fstage")
        fusedT = p_gate.tile([P, HN, S], F16, tag="fusedT", name="fusedT")
        for gi, (xTg, attnTg) in enumerate([(xT["en"], attn_enT), (xT["zh"], attn_zhT)]):
            g1T = p_gate.tile([P, HN, S], F16, tag="g1T", name="g1T")
            for m in range(HN):
                proj(2 * HN,
                     lambda k, m=m: ldT(sc["fw1"], m * P, P, k * P),
                     lambda k, f0, fl: (xTg[:, k, f0:f0 + fl] if k < HN
                                        else attnTg[:, k - HN, f0:f0 + fl]),
                     fsl_s,
                     lambda f0, fl, m=m: (fb1_r[0:1, m * P:(m + 1) * P], ones_row[0:1, :fl]),
                     lambda i, f0, fl, ps, m=m: nc.scalar.activation(
                         g1T[:, m, f0:f0 + fl], ps, AF.Relu))
            for m in range(HN):
                if gi == 0:
                    def epi0(i, f0, fl, ps, m=m):
                        nc.vector.tensor_scalar_mul(fstage[:, m, f0:f0 + fl], ps, 0.5)
                    bias = lambda f0, fl, m=m: (fb2x2[0:1, m * P:(m + 1) * P], ones_row[0:1, :fl])
                    epi = epi0
                else:
                    def epi1(i, f0, fl, ps, m=m):
                        nc.vector.scalar_tensor_tensor(fusedT[:, m, f0:f0 + fl], ps, 0.5,
                                                       fstage[:, m, f0:f0 + fl],
                                                       OP.mult, OP.add)
                    bias = lambda f0, fl, m=m: (fb2x2[0:1, m * P:(m + 1) * P], zero_row[0:1, :fl])
                    epi = epi1
                proj(HN,
                     lambda k, m=m: ldT(sc["fw2"], m * P, P, k * P),
                     lambda k, f0, fl: g1T[:, k, f0:f0 + fl],
                     fsl_s, bias, epi)

        # ---------------- node MHA + LN ----------------
        close_pool(p_attn_cm)
        p_mn_cm, p_mn = open_pool("p_mn")
        p_node_cm, p_node = open_pool("p_node", side="right")
        nodes_nat = mha(nqT, N, fusedT, sc["in_w_nd"], sc["out_w_nd"],
                        b_in["nd"], b_out["nd"], "nat", "nd", p_mn, p_node)
        close_pool(p_mn_cm)
        close_pool(p_gate_cm)
        close_pool(p_x_cm)
        ng_128 = bcast128(d["norm_g"], H, "ng", p_node)
        nb_128 = bcast128(d["norm_b"], H, "nb", p_node)
        lw2_128 = bcast128(d["len_w2"], H, "lw2", p_node)
        BSF = min(512, H)  # bn_stats max free
        for nn2 in range(NN):
            xg = nodes_nat[:, nn2, :].rearrange("p (a b) -> p a b", b=BSF)
            nsub = H // BSF
            stats = p_node.tile([P, nsub, 6], F32, tag="ln_stats", name="lns")
            for sgi in range(nsub):
                nc.vector.bn_stats(stats[:, sgi, :], xg[:, sgi, :])
            mv = p_node.tile([P, 2], F32, tag="ln_mv", name="lnmv")
            nc.vector.bn_aggr(mv[:], stats[:])
            nc.scalar.activation(mv[:, 1:2], mv[:, 1:2], AF.Sqrt, bias=eps_t[:])
            nc.vector.reciprocal(mv[:, 1:2], mv[:, 1:2])
            nc.vector.tensor_scalar(nodes_nat[:, nn2, :], nodes_nat[:, nn2, :],
                                    mv[:, 0:1], mv[:, 1:2], op0=OP.subtract, op1=OP.mult)
            nc.vector.tensor_tensor(nodes_nat[:, nn2, :], nodes_nat[:, nn2, :],
                                    ng_128[:], OP.mult)
            nc.vector.tensor_tensor(nodes_nat[:, nn2, :], nodes_nat[:, nn2, :],
                                    nb_128[:], OP.add)
            n16 = p_node.tile([P, H], F16, tag="n16", name="n16")
            nc.vector.tensor_copy(n16[:], nodes_nat[:, nn2, :])
            nc.sync.dma_start(sc["nodes"].ap()[nn2 * P:(nn2 + 1) * P, :], n16[:])
        nodesT = loadT_full(sc["nodes"], H, N, "nodesT", p_node)
        p_vq_cm, p_vq = open_pool("p_vq", side="right")

        # ---------------- VQ: scores, argmax, gather ----------------
        negnorm16 = p_vq.tile([1, K], F16, tag="negnorm16", name="negnorm16")
        dist = [p_vq.tile([P, K], F32, tag="dist%d" % i, name="dist%d" % i) for i in range(NN)]
        for kc in range(K // 512):
            cbts = []
            for hh in range(HN):
                cbt = wst.tile([P, 512], F16, tag="cbt")
                nc.sync.dma_start_transpose(cbt[:], sc["cb"].ap()[kc * 512:(kc + 1) * 512, hh * P:(hh + 1) * P])
                cbts.append(cbt)
            psn = psS.tile([1, 512], F32, tag="s")
            for hh in range(HN):
                sq = p_vq.tile([P, 512], F16, tag="cbsq", name="cbsq")
                nc.vector.tensor_tensor(sq[:], cbts[hh][:], cbts[hh][:], OP.mult)
                nc.tensor.matmul(psn[:], ones_col[:, 0:1], sq[:],
                                 start=(hh == 0), stop=(hh == HN - 1))
            nc.vector.tensor_scalar_mul(negnorm16[0:1, kc * 512:(kc + 1) * 512], psn[:], -0.5)
            for nn2 in range(NN):
                ps = psA.tile([P, 512], F32, tag="a", name="pd")
                for hh in range(HN):
                    nc.tensor.matmul(ps[:], nodesT[:, hh, nn2 * P:(nn2 + 1) * P], cbts[hh][:],
                                     start=(hh == 0), stop=False)
                nc.tensor.matmul(ps[:], ones_row[0:1, :P],
                                 negnorm16[0:1, kc * 512:(kc + 1) * 512],
                                 start=False, stop=True)
                nc.vector.tensor_copy(dist[nn2][:, kc * 512:(kc + 1) * 512], ps[:])
        qz_tiles = []
        vq_acc = p_node.tile([P, NN], F32, tag="vq_acc", name="vq_acc")
        for nn2 in range(NN):
            mx = p_vq.tile([P, 8], F32, tag="mx", name="mx")
            mi = p_vq.tile([P, 8], U32, tag="mi", name="mi")
            nc.vector.max(mx[:], dist[nn2][:])
            nc.vector.max_index(mi[:], mx[:], dist[nn2][:])
            ci = p_vq.tile([P, 1], I32, tag="ci%d" % nn2, name="ci")
            nc.vector.tensor_copy(ci[:], mi[:, 0:1])
            nc.sync.dma_start(d["codes"].ap().rearrange("(a b) -> a b", b=1)[nn2 * P:(nn2 + 1) * P, :], ci[:])
            qz = p_vq.tile([P, H], F32, tag="qz%d" % nn2, name="qz")
            nc.gpsimd.indirect_dma_start(
                out=qz[:], out_offset=None, in_=d["codebook"].ap(),
                in_offset=bass.IndirectOffsetOnAxis(ap=ci[:, :1], axis=0))
            qz_tiles.append(qz)
            nc.sync.dma_start(d["quantized"].ap()[nn2 * P:(nn2 + 1) * P, :], qz[:])
            q16 = p_vq.tile([P, H], F16, tag="q16", name="q16")
            nc.vector.tensor_copy(q16[:], qz[:])
            nc.sync.dma_start(sc["qz"].ap()[nn2 * P:(nn2 + 1) * P, :], q16[:])
            # vq partial: sum((nodes - qz)^2)
            diff = p_vq.tile([P, H], F32, tag="vqdiff", name="vqdiff")
            nc.vector.tensor_tensor(diff[:], nodes_nat[:, nn2, :], qz[:], OP.subtract)
            sqs = p_vq.tile([P, H], F32, tag="vqsq", name="vqsq")
            nc.scalar.activation(sqs[:], diff[:], AF.Square, accum_out=vq_acc[:, nn2:nn2 + 1])
        qzT = loadT_full(sc["qz"], H, N, "qzT", p_node)

        vqsum = p_node.tile([P, 1], F32, tag="vqsum", name="vqsum")
        if NN == 2:
            nc.vector.tensor_add(vqsum[:], vq_acc[:, 0:1], vq_acc[:, 1:2])
        else:
            nc.vector.tensor_reduce(vqsum[:], vq_acc[:], axis=mybir.AxisListType.X, op=OP.add)
        nc.sync.dma_start(sc_vqsum.ap(), vqsum[:])
        vqrow = p_node.tile([1, P], F32, tag="vqrow", name="vqrow")
        nc.sync.dma_start(vqrow[:], sc_vqsum.ap().rearrange("(a b) -> a b", a=1))
        vqtot = p_node.tile([1, 1], F32, tag="vqtot", name="vqtot")
        nc.vector.tensor_reduce(vqtot[:], vqrow[:], axis=mybir.AxisListType.X, op=OP.add)
        nc.vector.tensor_scalar_mul(vqtot[:], vqtot[:], float(1.0 / (B * N * H)))
        nc.sync.dma_start(d["vq_partial"].ap(), vqtot[:])

        # ---------------- length head ----------------
        fsl_h = fslices(H)
        t_nat = p_node.tile([P, NN, H], F32, tag="t_nat", name="t_nat")
        for nn2 in range(NN):
            proj(HN,
                 lambda k, nn2=nn2: nodesT[:, k, nn2 * P:(nn2 + 1) * P],
                 lambda k, f0, fl: ldT(sc["lw1"], f0, fl, k * P),
                 fsl_h,
                 lambda f0, fl: (ones_row[0:1, :P], lb1_r[0:1, f0:f0 + fl]),
                 lambda i, f0, fl, ps, nn2=nn2: nc.scalar.activation(
                     t_nat[:, nn2, f0:f0 + fl], ps, AF.Tanh))
            tmp = p_node.tile([P, H], F32, tag="lltmp", name="lltmp")
            nc.vector.tensor_tensor(tmp[:], t_nat[:, nn2, :], lw2_128[:], OP.mult)
            llc = p_node.tile([P, 1], F32, tag="llc", name="llc")
            nc.vector.tensor_reduce(llc[:], tmp[:], axis=mybir.AxisListType.X, op=OP.add)
            mk = p_node.tile([P, 1], U8, tag="mk", name="mk")
            nc.vector.tensor_tensor(mk[:], llc[:], nb2neg[:], OP.is_gt)
            nc.sync.dma_start(d["node_mask"].ap().rearrange("(a b) -> a b", b=1)[nn2 * P:(nn2 + 1) * P, :], mk[:])

        # ---------------- pointer logits ----------------
        def ptr_logits(pw_scr, emb_scr, ED, out_dram):
            pT = p_log.tile([P, HN, N], F16, tag="pT", name="pT")
            for m in range(HN):
                proj(HN,
                     lambda k, m=m: ldT(pw_scr, m * P, P, k * P),
                     lambda k, f0, fl: qzT[:, k, f0:f0 + fl],
                     fslices(N),
                     lambda f0, fl: (ones_row[0:1, :P], zero_row[0:1, :fl]),
                     lambda i, f0, fl, ps, m=m: nc.scalar.copy(pT[:, m, f0:f0 + fl], ps))
            for ec in range(ED // 512):
                pss = [psA.tile([P, 512], F32, tag="a", name="pp%d" % _i) for _i in range(NN)]
                for k in range(HN):
                    et = wst.tile([P, 512], F16, tag="wst512")
                    nc.sync.dma_start_transpose(et[:], emb_scr.ap()[ec * 512:(ec + 1) * 512, k * P:(k + 1) * P])
                    for nn2 in range(NN):
                        nc.tensor.matmul(pss[nn2][:], pT[:, k, nn2 * P:(nn2 + 1) * P], et[:],
                                         start=(k == 0), stop=(k == HN - 1))
                for nn2 in range(NN):
                    ot = p_log.tile([P, 512], F32, tag="ptr_out", name="ptro")
                    nc.vector.tensor_copy(ot[:], pss[nn2][:])
                    nc.sync.dma_start(out_dram.ap()[nn2 * P:(nn2 + 1) * P, ec * 512:(ec + 1) * 512], ot[:])

        close_pool(p_vq_cm)
        p_log_cm, p_log = open_pool("p_log", side="right")
        ptr_logits(sc["pw_ent"], sc["ent"], E, d["qid_logits"])
        ptr_logits(sc["pw_unit"], sc["unit"], U, d["unit_logits"])

        # ---------------- edge logits ----------------
        ew_t = []
        for hh in range(HN):
            nat = p_log.tile([P, H], F32, tag="ew_nat", name="ewnat")
            nc.sync.dma_start(nat[:], d["edge_w"].ap()[hh * P:(hh + 1) * P, :])
            e16 = p_log.tile([P, H], F16, tag="ew16_%d" % hh, name="ew16")
            nc.vector.tensor_copy(e16[:], nat[:])
            ew_t.append(e16)
        e1T = p_log.tile([P, HN, N], F16, tag="e1T", name="e1T")
        for g in range(HN):
            pfull = psA.tile([P, 512], F32, tag="a", name="pe")
            ps = pfull[:, :N]
            for hh in range(HN):
                nc.tensor.matmul(ps, ew_t[hh][:, g * P:(g + 1) * P], qzT[:, hh, :],
                                 start=(hh == 0), stop=(hh == HN - 1))
            nc.scalar.copy(e1T[:, g, :], ps)
        for nn2 in range(NN):
            pfull = psA.tile([P, 512], F32, tag="a", name="pe")
            ps = pfull[:, :N]
            for g in range(HN):
                nc.tensor.matmul(ps, e1T[:, g, nn2 * P:(nn2 + 1) * P], qzT[:, g, :],
                                 start=(g == 0), stop=(g == HN - 1))
            ebias = p_log.tile([P, N], F32, tag="ebias", name="ebias")
            nc.sync.dma_start(ebias[:], d["edge_bias"].ap()[nn2 * P:(nn2 + 1) * P, :])
            es = p_log.tile([P, N], F32, tag="es", name="es")
            nc.vector.tensor_tensor(es[:], ps, ebias[:], OP.add)
            nc.vector.tensor_scalar_add(es[:], es[:], eb_128[:, 0:1])
            nc.sync.dma_start(d["edge_logits"].ap()[nn2 * P:(nn2 + 1) * P, :], es[:])
        close_pool(p_log_cm)
        close_pool(p_node_cm)

    nc.finalize()
    return nc


def _get_program(dims):
    key = tuple(sorted(dims.items()))
    if key not in _BUILD_CACHE:
        _BUILD_CACHE[key] = build_program(**dims)
    return _BUILD_CACHE[key]


def kernel(**inputs):
    from concourse.bass_utils import run_bass_kernel_spmd

    B = inputs["h_en"].shape[0]
    dims = dict(FULL_DIMS)
    dims["B"] = B
    nc = _get_program(dims)

    names = ["h_en", "h_zh", "ent_embeds", "unit_embeds"]
    per_core = []
    for b in range(B):
        m = {
            "h_en": inputs["h_en"][b],
            "h_zh": inputs["h_zh"][b],
            "ent": inputs["ent_embeds"][b],
            "unit": inputs["unit_embeds"][b],
            "edge_b": np.asarray(inputs["edge_b"], np.float32).reshape(1),
            "len_b2": np.asarray(inputs["len_b2"], np.float32).reshape(1),
        }
        for k in ["en_in_w", "en_in_b", "en_out_w", "en_out_b",
                  "zh_in_w", "zh_in_b", "zh_out_w", "zh_out_b",
                  "nd_in_w", "nd_in_b", "nd_out_w", "nd_out_b",
                  "fuse_w1", "fuse_b1", "fuse_w2", "fuse_b2",
                  "node_queries", "norm_g", "norm_b", "codebook",
                  "ent_ptr_w", "unit_ptr_w", "edge_w", "edge_bias",
                  "len_w1", "len_b1", "len_w2"]:
            m[k] = inputs[k]
        m = {k: np.ascontiguousarray(np.asarray(v, dtype=np.float32)) for k, v in m.items()}
        per_core.append(m)

    trace = os.environ.get("KERNEL_TRACE") == "1"
    out = run_bass_kernel_spmd(nc, per_core, core_ids=list(range(B)), trace=trace)
    global LAST_RESULT
    LAST_RESULT = out
    rs = out.results

    quantized = np.stack([r["quantized"] for r in rs])
    codes = np.stack([r["codes"] for r in rs]).astype(np.int32)
    qid = np.stack([r["qid_logits"] for r in rs])
    unit = np.stack([r["unit_logits"] for r in rs])
    edge = np.stack([r["edge_logits"] for r in rs])
    mask = np.stack([r["node_mask"] for r in rs]).astype(bool)
    vq = np.float32(sum(float(r["vq_partial"][0, 0]) for r in rs))

    # apply candidate masks (no-ops when masks are all ones, per spec)
    ent_mask = np.asarray(inputs["ent_mask"], bool)
    unit_mask = np.asarray(inputs["unit_mask"], bool)
    if not ent_mask.all():
        qid = np.where(ent_mask[:, None, :], qid, -np.inf).astype(np.float32)
    if not unit_mask.all():
        unit = np.where(unit_mask[:, None, :], unit, -np.inf).astype(np.float32)

    return quantized, codes, qid, unit, edge, mask, np.asarray(vq, np.float32)
